# revision 3
# baseline (speedup 1.0000x reference)
"""Causal self-attention on 8 trn2 NeuronCores — v3 (software-pipelined).

Sharding: DP4 (batch) x TP2 (head groups of 8). Core c -> batch c//2,
head group c%2. Each core computes qkv^T for its 512 channels, causal
attention for its 8 heads over all T=2048 queries, and a partial
projection y_partial = O_g @ W_proj[rows_g] (+ folded bias on group 0).
Host sums the two partials per batch and transposes (kernel emits y^T).

v3 over v2:
- PE is in-order; v2 stalled ~126us waiting on Act exp / DVE normalize.
  v3 interleaves qkv chains for slice s+1 and proj chains for slice s-1
  as "filler" matmul pairs between attention block iterations, and
  reorders each block to [scores(i+1), PV(i)] so exp(i) overlaps PE.
- V-bias folded into b_proj on host (kills 32 DVE tensor_scalar_adds).
- Normalize reads PV PSUM directly (recip of row 64, broadcast, mul) —
  no [65,512] staging copy.
- W_proj and O in bf16 (same PE rate, half DMA/SBUF).
"""
import sys

sys.path.insert(0, "/opt/trn_rl_repo")

from collections import deque

import numpy as np

import concourse.bass as bass
import concourse.tile as tile
from concourse import bacc, mybir

f32 = mybir.dt.float32
f32r = mybir.dt.float32r
bf16 = mybir.dt.bfloat16
AFT = mybir.ActivationFunctionType
ALU = mybir.AluOpType

N_CORES = 8
B, T, C = 4, 2048, 1024
H, HD = 16, 64            # total heads, head dim
HPC = 8                   # heads per core
CPC = 512                 # channels per core (q, k or v)
NT = T // 128             # 16 t-tiles of 128
NS = T // 512             # 4 t-slices of 512
NC_T = C // 128           # 8 C-tiles (contraction)
SCALE = 1.0 / np.sqrt(HD)

FILL_CAP = 4              # max filler pair-steps pumped per B iteration


def build_nc(repeat: int = 1):
    nc = bacc.Bacc("TRN2", target_bir_lowering=False, debug=False,
                   num_devices=N_CORES)

    xb_d = nc.dram_tensor("xb", [C, T], f32, kind="ExternalInput")
    wqkv_d = nc.dram_tensor("wqkv", [C, 3 * CPC], f32, kind="ExternalInput")
    bqkv_d = nc.dram_tensor("bqkv", [128, 8], f32, kind="ExternalInput")
    wp_d = nc.dram_tensor("wp", [CPC, C], bf16, kind="ExternalInput")
    bp_d = nc.dram_tensor("bp", [128, 8], f32, kind="ExternalInput")
    yt_d = nc.dram_tensor("yT", [C, T], f32, kind="ExternalOutput")

    with tile.TileContext(nc) as tc:
        def body(_=None):
            _build_body(nc, tc, xb_d, wqkv_d, bqkv_d, wp_d, bp_d, yt_d)
        if repeat == 1:
            body()
        else:
            with tc.For_i(0, repeat, 1):
                body()
    nc.compile()
    return nc


class _Fill:
    """FIFO of emission generators, pumped step-by-step between B iters."""

    def __init__(self):
        self.q = deque()
        self.steps = 0

    def push(self, gen, nsteps):
        self.q.append(gen)
        self.steps += nsteps

    def pump(self, n):
        while n > 0 and self.q:
            try:
                next(self.q[0])
                self.steps = max(0, self.steps - 1)
                n -= 1
            except StopIteration:
                self.q.popleft()

    def drain(self):
        while self.q:
            try:
                next(self.q[0])
            except StopIteration:
                self.q.popleft()
        self.steps = 0


class _St:
    pass


def _run_gen(g):
    for _ in g:
        pass


def _build_body(nc, tc, xb_d, wqkv_d, bqkv_d, wp_d, bp_d, yt_d):
    st = _St()
    st.nc = nc

    pers_cm = tc.tile_pool(name="pers", bufs=1)
    pers = pers_cm.__enter__()

    st.bqkv = pers.tile([128, 8], f32, name="bqkv")
    nc.sync.dma_start(st.bqkv[:], bqkv_d.ap())
    st.bp = pers.tile([128, 8], f32, name="bp")
    nc.sync.dma_start(st.bp[:], bp_d.ap())

    # weights (Act queue so xb DMAs on sync aren't delayed)
    st.wqkv = [pers.tile([128, 3 * CPC], f32r, name=f"wqkv{ci}")
               for ci in range(NC_T)]
    for ci in range(NC_T):
        nc.scalar.dma_start(
            st.wqkv[ci][:],
            wqkv_d.ap()[128 * ci:128 * ci + 128, :].bitcast(f32r))
    st.wp = [pers.tile([128, C], bf16, name=f"wp{i}") for i in range(4)]
    for ci in range(4):
        nc.scalar.dma_start(
            st.wp[ci][:], wp_d.ap()[128 * ci:128 * ci + 128, :])

    # qkv^T results
    st.qt = [pers.tile([128, T], f32r, name=f"qt{i}") for i in range(4)]
    st.kt = [pers.tile([128, T], f32r, name=f"kt{i}") for i in range(4)]
    # V natural + ones col per head (bf16); only col 64 needs the memset
    st.vaug = [pers.tile([128, 8 * 65], bf16, name=f"vaug{i}")
               for i in range(NT)]
    for i in range(NT):
        ones = st.vaug[i][:].rearrange("p (h w) -> p h w", w=65)[:, :, 64:65]
        nc.gpsimd.memset(ones, 1.0)

    st.yt_d = yt_d
    st.fill = _Fill()

    with tc.tile_pool(name="xt", bufs=12) as xt_pool, \
         tc.tile_pool(name="pt", bufs=4) as pt_pool, \
         tc.tile_pool(name="rl", bufs=2) as rl_pool, \
         tc.tile_pool(name="rlb", bufs=2) as rlb_pool, \
         tc.tile_pool(name="otp", bufs=8) as ot_pool, \
         tc.tile_pool(name="ytp", bufs=2) as yt_pool, \
         tc.tile_pool(name="pac", bufs=2, space="PSUM") as pac_pool, \
         tc.tile_pool(name="pst", bufs=2, space="PSUM") as pst_pool, \
         tc.tile_pool(name="ots", bufs=2, space="PSUM") as ots_pool:

        st.xt, st.pt, st.rl, st.rlb = xt_pool, pt_pool, rl_pool, rlb_pool
        st.ot_pool, st.yt_pool = ot_pool, yt_pool
        st.pac, st.pst, st.ots = pac_pool, pst_pool, ots_pool

        ot_tiles = {}

        # prologue: A(0) emitted directly
        xts0 = _xt_dmas(st, 0, xb_d)
        for g in _a_qk_units(st, 0, xts0) + _a_v_units(st, 0, xts0):
            _run_gen(g)

        for s in range(NS):
            # queue fillers for this B(s) segment: qkv chains for slice
            # s+1, then projection chains for slice s-1
            if s + 1 < NS:
                xts_n = _xt_dmas(st, s + 1, xb_d)
                for g in _a_qk_units(st, s + 1, xts_n):
                    st.fill.push(g, 10)
                for g in _a_v_units(st, s + 1, xts_n):
                    st.fill.push(g, 10)
            if s >= 1:
                for g in _c_units(st, s - 1, ot_tiles):
                    st.fill.push(g, 6)

            _phase_b(st, s, ot_tiles)
            st.fill.drain()

        # epilogue: C(3)
        for g in _c_units(st, NS - 1, ot_tiles):
            _run_gen(g)

    pers_cm.__exit__(None, None, None)


def _xt_dmas(st, s, xb_d):
    nc = st.nc
    xts = []
    for ci in range(NC_T):
        xtt = st.xt.tile([128, 512], f32r, name="xt")
        nc.sync.dma_start(
            xtt[:],
            xb_d.ap()[128 * ci:128 * ci + 128,
                      512 * s:512 * s + 512].bitcast(f32r))
        xts.append(xtt)
    return xts


def _a_qk_units(st, s, xts):
    return [_a_qk_unit(st, s, gp, xts) for gp in range(4)]


def _a_v_units(st, s, xts):
    return [_a_v_unit(st, s, tp, xts) for tp in range(2)]


def _a_qk_unit(st, s, gp, xts):
    """Q (g<4) / K (g>=4) chains for t-slice s: out[c_out 128, t 512]."""
    nc = st.nc
    g0, g1 = 2 * gp, 2 * gp + 1
    ps0 = st.pac.tile([128, 512], f32, name="pac")
    ps1 = st.pac.tile([128, 512], f32, name="pac")
    for ci in range(NC_T):
        nc.tensor.matmul(ps0[:],
                         st.wqkv[ci][:, 128 * g0:128 * g0 + 128], xts[ci][:],
                         start=(ci == 0), stop=(ci == NC_T - 1))
        nc.tensor.matmul(ps1[:],
                         st.wqkv[ci][:, 128 * g1:128 * g1 + 128], xts[ci][:],
                         start=(ci == 0), stop=(ci == NC_T - 1))
        yield
    for src, g in ((ps0, g0), (ps1, g1)):
        if g < 4:
            dst = st.qt[g][:, 512 * s:512 * s + 512]
            # bias pre-scaled on host: (ps*SCALE) + bias
            nc.vector.tensor_scalar(dst, src[:], SCALE,
                                    st.bqkv[:, g:g + 1],
                                    ALU.mult, ALU.add)
        else:
            dst = st.kt[g - 4][:, 512 * s:512 * s + 512]
            nc.vector.tensor_scalar_add(dst, src[:], st.bqkv[:, g:g + 1])
        yield


def _a_v_unit(st, s, tp, xts):
    """V chains for t-slice s: out[t 128, c_v 512], two t-tiles."""
    nc = st.nc
    ps0 = st.pac.tile([128, 512], f32, name="pac")
    ps1 = st.pac.tile([128, 512], f32, name="pac")
    for ci in range(NC_T):
        for ps, tt in ((ps0, 2 * tp), (ps1, 2 * tp + 1)):
            nc.tensor.matmul(
                ps[:],
                xts[ci][:, 128 * tt:128 * tt + 128],
                st.wqkv[ci][:, 1024:1536],
                start=(ci == 0), stop=(ci == NC_T - 1))
        yield
    for ps, half in ((ps0, 0), (ps1, 1)):
        ti = 4 * s + 2 * tp + half
        dst = st.vaug[ti][:].rearrange("p (h w) -> p h w", w=65)[:, :, 0:64]
        nc.vector.tensor_copy(
            dst, ps[:].rearrange("p (h w) -> p h w", w=64))
        yield


def _c_units(st, s, ot_tiles):
    return [_c_unit(st, s, gp, ot_tiles) for gp in range(4)]


def _c_unit(st, s, gp, ot_tiles):
    """Projection for t-slice s: yT[:, cols s] = sum_ci wp[ci].T @ O^T."""
    nc = st.nc
    g0, g1 = 2 * gp, 2 * gp + 1
    ps0 = st.pac.tile([128, 512], f32, name="pac")
    ps1 = st.pac.tile([128, 512], f32, name="pac")
    for ci in range(4):
        nc.tensor.matmul(ps0[:],
                         st.wp[ci][:, 128 * g0:128 * g0 + 128],
                         ot_tiles[(ci, s)][:],
                         start=(ci == 0), stop=(ci == 3))
        nc.tensor.matmul(ps1[:],
                         st.wp[ci][:, 128 * g1:128 * g1 + 128],
                         ot_tiles[(ci, s)][:],
                         start=(ci == 0), stop=(ci == 3))
        yield
    for ps, g in ((ps0, g0), (ps1, g1)):
        yt = st.yt_pool.tile([128, 512], f32, name="yt")
        nc.vector.tensor_scalar_add(yt[:], ps[:], st.bp[:, g:g + 1])
        nc.sync.dma_start(
            st.yt_d.ap()[128 * g:128 * g + 128, 512 * s:512 * s + 512],
            yt[:])
        yield


def _phase_b(st, j, ot_tiles):
    """Attention for q-tile j (512 queries), 4 head pairs, with fillers."""
    nc = st.nc
    i_max = 4 * j + 3
    iters_left = 4 * (i_max + 1)

    for hp in range(4):
        hA, hB = 2 * hp, 2 * hp + 1
        oa = st.ots.tile([65, 512], f32, name="ots")
        ob = st.ots.tile([65, 512], f32, name="ots")
        ptiles = {}

        def sc(i):
            o = i - 4 * j
            # valid query columns start at 128*o within this 512-q block;
            # keep the score matmul at N>=256 (f32r full-rate floor)
            q0v = max(0, 128 * o)
            q0 = min(q0v, 256)
            ps = st.pst.tile([128, 1024], f32, name="pst")
            # packed score pair: head A on rows 0-63 (T0), B on 64-127 (T8)
            nc.tensor.matmul(ps[:, q0:512],
                             st.kt[hp][0:64, 128 * i:128 * i + 128],
                             st.qt[hp][0:64, 512 * j + q0:512 * j + 512],
                             start=True, stop=True, tile_position=(0, 0))
            nc.tensor.matmul(ps[:, 512 + q0:1024],
                             st.kt[hp][64:128, 128 * i:128 * i + 128],
                             st.qt[hp][64:128, 512 * j + q0:512 * j + 512],
                             start=True, stop=True, tile_position=(64, 0))
            ptile = st.pt.tile([128, 1024], bf16, name="pt")
            if q0v == 0:
                nc.scalar.activation(ptile[:], ps[:], AFT.Exp)
            else:
                # two contiguous exps (strided 3D activation is slow)
                nc.scalar.activation(ptile[:, q0v:512], ps[:, q0v:512],
                                     AFT.Exp)
                nc.scalar.activation(ptile[:, 512 + q0v:1024],
                                     ps[:, 512 + q0v:1024], AFT.Exp)
            if o >= 0:
                # zero the strict upper triangle of the diagonal 128x128
                # sub-block: keep where (q - k) >= 0, q local to the block
                tri = ptile[:].rearrange("p (g q) -> p g q",
                                         q=512)[:, :, q0v:q0v + 128]
                nc.gpsimd.affine_select(
                    out=tri, in_=tri, compare_op=ALU.is_ge, fill=0.0,
                    base=0, channel_multiplier=-1,
                    pattern=[[0, 2], [1, 128]])
            ptiles[i] = (ptile, q0v)

        sc(0)
        for i in range(i_max + 1):
            if i + 1 <= i_max:
                sc(i + 1)
            ptile, q0v = ptiles.pop(i)
            nc.tensor.matmul(oa[:, q0v:512],
                             st.vaug[i][:, 65 * hA:65 * hA + 65],
                             ptile[:, q0v:512],
                             start=(i == 0), stop=(i == i_max))
            nc.tensor.matmul(ob[:, q0v:512],
                             st.vaug[i][:, 65 * hB:65 * hB + 65],
                             ptile[:, 512 + q0v:1024],
                             start=(i == 0), stop=(i == i_max))
            if iters_left > 1:
                k = -(-st.fill.steps // iters_left)
                st.fill.pump(min(k, FILL_CAP))
            iters_left -= 1

        # normalize straight out of PSUM: r = 1/rowsum, O = O_un * r
        ot = st.ot_pool.tile([128, 512], bf16, name="ot")
        with nc.allow_low_precision(reason="bf16 attention output"):
            for hl, po in ((0, oa), (1, ob)):
                rl = st.rl.tile([1, 512], f32, name="rl")
                nc.vector.reciprocal(rl[:], po[64:65, :])
                rlb = st.rlb.tile([64, 512], f32, name="rlb")
                nc.gpsimd.partition_broadcast(rlb[:], rl[:])
                nc.vector.tensor_mul(ot[64 * hl:64 * hl + 64, :],
                                     po[0:64, :], rlb[:])
        ot_tiles[(hp, j)] = ot


def make_inputs(x, W_attn, b_attn, W_proj, b_proj):
    """Host-side sharding: per-core input dicts."""
    import ml_dtypes
    x = np.asarray(x, np.float32)
    W_attn = np.asarray(W_attn, np.float32)
    b_attn = np.asarray(b_attn, np.float32)
    W_proj = np.asarray(W_proj, np.float32)
    b_proj = np.asarray(b_proj, np.float32)

    in_maps = []
    for core in range(N_CORES):
        b, g = divmod(core, 2)
        cols = np.concatenate([
            np.arange(CPC * g, CPC * g + CPC),
            C + np.arange(CPC * g, CPC * g + CPC),
            2 * C + np.arange(CPC * g, CPC * g + CPC)])
        wqkv = np.ascontiguousarray(W_attn[:, cols])
        bqk = b_attn[cols[:2 * CPC]].copy()           # q,k bias [1024]
        bqk[:CPC] *= SCALE                            # fold q-scale into bias
        bqkv = np.ascontiguousarray(bqk.reshape(8, 128).T)
        wp_f = W_proj[CPC * g:CPC * g + CPC, :]
        wp = np.ascontiguousarray(wp_f.astype(ml_dtypes.bfloat16))
        # fold the attention V-bias through the projection:
        # y += b_attn_v[g] @ W_proj[rows_g]  (+ b_proj once, on group 0)
        bv = b_attn[2 * C + CPC * g:2 * C + CPC * g + CPC]
        bpv = bv @ wp_f + (b_proj if g == 0 else 0.0)
        bpv = np.ascontiguousarray(bpv.astype(np.float32).reshape(8, 128).T)
        in_maps.append({
            "xb": np.ascontiguousarray(x[b].T),
            "wqkv": wqkv,
            "bqkv": bqkv,
            "wp": wp,
            "bp": bpv,
        })
    return in_maps


def unshard(results):
    """Combine per-core yT partials into [B, T, C] output."""
    out = np.empty((B, T, C), np.float32)
    for b in range(B):
        yt = results[2 * b]["yT"] + results[2 * b + 1]["yT"]
        out[b] = yt.T
    return out


_nc_cache = {}


def kernel(x, W_attn, b_attn, W_proj, b_proj):
    from concourse.bass_utils import run_bass_kernel_spmd
    if "nc" not in _nc_cache:
        _nc_cache["nc"] = build_nc(repeat=1)
    nc = _nc_cache["nc"]
    in_maps = make_inputs(x, W_attn, b_attn, W_proj, b_proj)
    res = run_bass_kernel_spmd(nc, in_maps, core_ids=list(range(N_CORES)),
                               trace=False)
    return unshard(res.results)


# revision 14
# speedup vs baseline: 1.8914x; 1.8914x over previous
"""Causal self-attention on 8 trn2 NeuronCores — v3 (software-pipelined).

Sharding: DP4 (batch) x TP2 (head groups of 8). Core c -> batch c//2,
head group c%2. Each core computes qkv^T for its 512 channels, causal
attention for its 8 heads over all T=2048 queries, and a partial
projection y_partial = O_g @ W_proj[rows_g] (+ folded bias on group 0).
Host sums the two partials per batch and transposes (kernel emits y^T).

v3 over v2:
- PE is in-order; v2 stalled ~126us waiting on Act exp / DVE normalize.
  v3 interleaves qkv chains for slice s+1 and proj chains for slice s-1
  as "filler" matmul pairs between attention block iterations, and
  reorders each block to [scores(i+1), PV(i)] so exp(i) overlaps PE.
- V-bias folded into b_proj on host (kills 32 DVE tensor_scalar_adds).
- Normalize reads PV PSUM directly (recip of row 64, broadcast, mul) —
  no [65,512] staging copy.
- W_proj and O in bf16 (same PE rate, half DMA/SBUF).
"""
import sys

sys.path.insert(0, "/opt/trn_rl_repo")

from collections import deque

import numpy as np

import concourse.bass as bass
import concourse.tile as tile
from concourse import bacc, mybir

f32 = mybir.dt.float32
f32r = mybir.dt.float32r
bf16 = mybir.dt.bfloat16
AFT = mybir.ActivationFunctionType
ALU = mybir.AluOpType

N_CORES = 8
B, T, C = 4, 2048, 1024
H, HD = 16, 64            # total heads, head dim
HPC = 8                   # heads per core
CPC = 512                 # channels per core (q, k or v)
NT = T // 128             # 16 t-tiles of 128
NS = T // 512             # 4 t-slices of 512
NC_T = C // 128           # 8 C-tiles (contraction)
SCALE = 1.0 / np.sqrt(HD)

FILL_CAP = 4              # max filler pair-steps pumped per B iteration


def build_nc(repeat: int = 1):
    nc = bacc.Bacc("TRN2", target_bir_lowering=False, debug=False,
                   num_devices=N_CORES)

    xb_d = nc.dram_tensor("xb", [C, T], f32, kind="ExternalInput")
    wqkv_d = nc.dram_tensor("wqkv", [C, 3 * CPC], f32, kind="ExternalInput")
    bqkv_d = nc.dram_tensor("bqkv", [128, 8], f32, kind="ExternalInput")
    wp_d = nc.dram_tensor("wp", [CPC, C], bf16, kind="ExternalInput")
    bp_d = nc.dram_tensor("bp", [128, 8], f32, kind="ExternalInput")
    yt_d = nc.dram_tensor("yT", [C, T], f32, kind="ExternalOutput")

    with tile.TileContext(nc) as tc:
        def body(_=None):
            _build_body(nc, tc, xb_d, wqkv_d, bqkv_d, wp_d, bp_d, yt_d)
        if repeat == 1:
            body()
        else:
            with tc.For_i(0, repeat, 1):
                body()
    nc.compile()
    return nc


class _Fill:
    """FIFO of emission generators, pumped step-by-step between B iters."""

    def __init__(self):
        self.q = deque()
        self.steps = 0

    def push(self, gen, nsteps):
        self.q.append(gen)
        self.steps += nsteps

    def pump(self, n):
        while n > 0 and self.q:
            try:
                next(self.q[0])
                self.steps = max(0, self.steps - 1)
                n -= 1
            except StopIteration:
                self.q.popleft()

    def drain(self):
        while self.q:
            try:
                next(self.q[0])
            except StopIteration:
                self.q.popleft()
        self.steps = 0


class _St:
    pass


def _run_gen(g):
    for _ in g:
        pass


def _build_body(nc, tc, xb_d, wqkv_d, bqkv_d, wp_d, bp_d, yt_d):
    st = _St()
    st.nc = nc

    pers_cm = tc.tile_pool(name="pers", bufs=1)
    pers = pers_cm.__enter__()

    # all startup DMAs ride the Act queue: its last per-iteration work (the
    # final exps) ends well before the sync queue's (trailing yT stores), so
    # under For_i the next iteration's weight/x loads issue ~10us earlier
    st.bqkv = pers.tile([128, 8], f32, name="bqkv")
    nc.scalar.dma_start(st.bqkv[:], bqkv_d.ap())
    st.bp = pers.tile([128, 8], f32, name="bp")
    nc.scalar.dma_start(st.bp[:], bp_d.ap())

    # weight tiles; DMAs are emitted interleaved with the slice-0 x loads
    # inside the pool block (see below) so the first qkv chain starts early
    st.wqkv = [pers.tile([128, 3 * CPC], f32r, name=f"wqkv{ci}")
               for ci in range(NC_T)]
    st.wp = [pers.tile([128, C], bf16, name=f"wp{i}") for i in range(4)]

    # qkv^T results
    st.qt = [pers.tile([128, T], f32r, name=f"qt{i}") for i in range(4)]
    st.kt = [pers.tile([128, T], f32r, name=f"kt{i}") for i in range(4)]
    # V natural + ones col per head (bf16); only col 64 needs the memset
    st.vaug = [pers.tile([128, 8 * 65], bf16, name=f"vaug{i}")
               for i in range(NT)]
    for i in range(NT):
        ones = st.vaug[i][:].rearrange("p (h w) -> p h w", w=65)[:, :, 64:65]
        nc.gpsimd.memset(ones, 1.0)

    # causal mask for the diagonal 128x128 sub-block, duplicated for the
    # two heads of a pair: 0 where q >= k, -1e30 above the diagonal
    st.trimask = pers.tile([128, 256], f32, name="trimask")
    nc.gpsimd.memset(st.trimask[:], 0.0)
    tri = st.trimask[:].rearrange("p (g q) -> p g q", q=128)
    nc.gpsimd.affine_select(
        out=tri, in_=tri, compare_op=ALU.is_ge, fill=-1e30,
        base=0, channel_multiplier=-1, pattern=[[0, 2], [1, 128]])

    st.yt_d = yt_d
    st.fill = _Fill()

    with tc.tile_pool(name="xt", bufs=12) as xt_pool, \
         tc.tile_pool(name="pt", bufs=4) as pt_pool, \
         tc.tile_pool(name="rl", bufs=2) as rl_pool, \
         tc.tile_pool(name="rlb", bufs=2) as rlb_pool, \
         tc.tile_pool(name="otp", bufs=8) as ot_pool, \
         tc.tile_pool(name="ytp", bufs=2) as yt_pool, \
         tc.tile_pool(name="pac", bufs=2, space="PSUM") as pac_pool, \
         tc.tile_pool(name="pst", bufs=2, space="PSUM") as pst_pool, \
         tc.tile_pool(name="ots", bufs=2, space="PSUM") as ots_pool:

        st.xt, st.pt, st.rl, st.rlb = xt_pool, pt_pool, rl_pool, rlb_pool
        st.ot_pool, st.yt_pool = ot_pool, yt_pool
        st.pac, st.pst, st.ots = pac_pool, pst_pool, ots_pool

        ot_tiles = {}

        # prologue: A(0) emitted directly. All startup loads ride the Act
        # queue (prefetches at the prior iteration's tail under For_i),
        # interleaved wqkv[ci]/x[ci] so the first qkv chain starts early.
        xts0 = []
        for ci in range(NC_T):
            xtt = xt_pool.tile([128, 512], f32r, name="xt")
            nc.scalar.dma_start(
                st.wqkv[ci][:],
                wqkv_d.ap()[128 * ci:128 * ci + 128, :].bitcast(f32r))
            nc.scalar.dma_start(
                xtt[:],
                xb_d.ap()[128 * ci:128 * ci + 128, 0:512].bitcast(f32r))
            xts0.append(xtt)
        for ci in range(4):
            nc.scalar.dma_start(
                st.wp[ci][:], wp_d.ap()[128 * ci:128 * ci + 128, :])
        for g in _a_qk_units(st, 0, xts0) + _a_v_units(st, 0, xts0):
            _run_gen(g)

        for s in range(NS):
            # queue fillers for this B(s) segment: qkv chains for slice
            # s+1, then projection chains for slice s-1
            if s + 1 < NS:
                xts_n = _xt_dmas(st, s + 1, xb_d)
                for g in _a_qk_units(st, s + 1, xts_n):
                    st.fill.push(g, 10)
                for g in _a_v_units(st, s + 1, xts_n):
                    st.fill.push(g, 10)
            if s >= 1:
                for g in _c_units(st, s - 1, ot_tiles):
                    st.fill.push(g, 6)

            _phase_b(st, s, ot_tiles)
            st.fill.drain()

        # epilogue: C(3)
        for g in _c_units(st, NS - 1, ot_tiles):
            _run_gen(g)

    pers_cm.__exit__(None, None, None)


def _xt_dmas(st, s, xb_d, queue=None):
    nc = st.nc
    q = queue if queue is not None else nc.sync
    xts = []
    for ci in range(NC_T):
        xtt = st.xt.tile([128, 512], f32r, name="xt")
        q.dma_start(
            xtt[:],
            xb_d.ap()[128 * ci:128 * ci + 128,
                      512 * s:512 * s + 512].bitcast(f32r))
        xts.append(xtt)
    return xts


def _a_qk_units(st, s, xts):
    return [_a_qk_unit(st, s, gp, xts) for gp in range(4)]


def _a_v_units(st, s, xts):
    return [_a_v_unit(st, s, tp, xts) for tp in range(2)]


def _a_qk_unit(st, s, gp, xts):
    """Q (g<4) / K (g>=4) chains for t-slice s: out[c_out 128, t 512]."""
    nc = st.nc
    g0, g1 = 2 * gp, 2 * gp + 1
    ps0 = st.pac.tile([128, 512], f32, name="pac")
    ps1 = st.pac.tile([128, 512], f32, name="pac")
    for ci in range(NC_T):
        nc.tensor.matmul(ps0[:],
                         st.wqkv[ci][:, 128 * g0:128 * g0 + 128], xts[ci][:],
                         start=(ci == 0), stop=(ci == NC_T - 1))
        nc.tensor.matmul(ps1[:],
                         st.wqkv[ci][:, 128 * g1:128 * g1 + 128], xts[ci][:],
                         start=(ci == 0), stop=(ci == NC_T - 1))
        yield
    for src, g in ((ps0, g0), (ps1, g1)):
        if g < 4:
            dst = st.qt[g][:, 512 * s:512 * s + 512]
            # bias pre-scaled on host: (ps*SCALE) + bias
            nc.vector.tensor_scalar(dst, src[:], SCALE,
                                    st.bqkv[:, g:g + 1],
                                    ALU.mult, ALU.add)
        else:
            dst = st.kt[g - 4][:, 512 * s:512 * s + 512]
            nc.vector.tensor_scalar_add(dst, src[:], st.bqkv[:, g:g + 1])
        yield


def _a_v_unit(st, s, tp, xts):
    """V chains for t-slice s: out[t 128, c_v 512], two t-tiles."""
    nc = st.nc
    ps0 = st.pac.tile([128, 512], f32, name="pac")
    ps1 = st.pac.tile([128, 512], f32, name="pac")
    for ci in range(NC_T):
        for ps, tt in ((ps0, 2 * tp), (ps1, 2 * tp + 1)):
            nc.tensor.matmul(
                ps[:],
                xts[ci][:, 128 * tt:128 * tt + 128],
                st.wqkv[ci][:, 1024:1536],
                start=(ci == 0), stop=(ci == NC_T - 1))
        yield
    for ps, half in ((ps0, 0), (ps1, 1)):
        ti = 4 * s + 2 * tp + half
        dst = st.vaug[ti][:].rearrange("p (h w) -> p h w", w=65)[:, :, 0:64]
        nc.vector.tensor_copy(
            dst, ps[:].rearrange("p (h w) -> p h w", w=64))
        yield


def _c_units(st, s, ot_tiles):
    return [_c_unit(st, s, gp, ot_tiles) for gp in range(4)]


def _c_unit(st, s, gp, ot_tiles):
    """Projection for t-slice s: yT[:, cols s] = sum_ci wp[ci].T @ O^T."""
    nc = st.nc
    g0, g1 = 2 * gp, 2 * gp + 1
    ps0 = st.pac.tile([128, 512], f32, name="pac")
    ps1 = st.pac.tile([128, 512], f32, name="pac")
    for ci in range(4):
        nc.tensor.matmul(ps0[:],
                         st.wp[ci][:, 128 * g0:128 * g0 + 128],
                         ot_tiles[(ci, s)][:],
                         start=(ci == 0), stop=(ci == 3))
        nc.tensor.matmul(ps1[:],
                         st.wp[ci][:, 128 * g1:128 * g1 + 128],
                         ot_tiles[(ci, s)][:],
                         start=(ci == 0), stop=(ci == 3))
        yield
    for ps, g in ((ps0, g0), (ps1, g1)):
        yt = st.yt_pool.tile([128, 512], f32, name="yt")
        nc.vector.tensor_scalar_add(yt[:], ps[:], st.bp[:, g:g + 1])
        nc.sync.dma_start(
            st.yt_d.ap()[128 * g:128 * g + 128, 512 * s:512 * s + 512],
            yt[:])
        yield


def _phase_b(st, j, ot_tiles):
    """Attention for q-tile j (512 queries), 4 head pairs, with fillers."""
    nc = st.nc
    i_max = 4 * j + 3
    iters_left = 4 * (i_max + 1)

    for hp in range(4):
        hA, hB = 2 * hp, 2 * hp + 1
        oa = st.ots.tile([65, 512], f32, name="ots")
        ob = st.ots.tile([65, 512], f32, name="ots")
        ptiles = {}

        def sc(i):
            o = i - 4 * j
            # valid query columns start at 128*o within this 512-q block;
            # compute the full 512 anyway: uniform full-width MMs + a single
            # full-width exp beat the ragged-edge clamping on HW
            q0v = max(0, 128 * o)
            ps = st.pst.tile([128, 1024], f32, name="pst")
            # packed score pair: head A on rows 0-63 (T0), B on 64-127 (T8)
            nc.tensor.matmul(ps[:, 0:512],
                             st.kt[hp][0:64, 128 * i:128 * i + 128],
                             st.qt[hp][0:64, 512 * j:512 * j + 512],
                             start=True, stop=True, tile_position=(0, 0))
            nc.tensor.matmul(ps[:, 512:1024],
                             st.kt[hp][64:128, 128 * i:128 * i + 128],
                             st.qt[hp][64:128, 512 * j:512 * j + 512],
                             start=True, stop=True, tile_position=(64, 0))
            if o >= 0:
                # add the -1e30 upper-triangle mask to the diagonal 128x128
                # sub-block IN PSUM (exp -> 0), keeping the mask off the
                # exp->PV critical chain (Pool overlaps the prior exp)
                tri = ps[:].rearrange("p (g q) -> p g q",
                                      q=512)[:, :, q0v:q0v + 128]
                trim = st.trimask[:].rearrange("p (g q) -> p g q", q=128)
                nc.vector.tensor_add(tri, tri, trim)
            # single full-width exp; columns left of q0v hold stale/dead
            # scores whose exps are finite and never read by PV
            ptile = st.pt.tile([128, 1024], bf16, name="pt")
            nc.scalar.activation(ptile[:], ps[:], AFT.Exp)
            ptiles[i] = (ptile, q0v)

        sc(0)
        for i in range(i_max + 1):
            if i + 1 <= i_max:
                sc(i + 1)
            ptile, q0v = ptiles.pop(i)
            nc.tensor.matmul(oa[:, q0v:512],
                             st.vaug[i][:, 65 * hA:65 * hA + 65],
                             ptile[:, q0v:512],
                             start=(i == 0), stop=(i == i_max))
            nc.tensor.matmul(ob[:, q0v:512],
                             st.vaug[i][:, 65 * hB:65 * hB + 65],
                             ptile[:, 512 + q0v:1024],
                             start=(i == 0), stop=(i == i_max))
            if iters_left > 1:
                k = -(-st.fill.steps // iters_left)
                st.fill.pump(min(k, FILL_CAP))
            iters_left -= 1

        # normalize straight out of PSUM: r = 1/rowsum, O = O_un * r;
        # the two heads' chains are interleaved so DVE/Pool overlap
        ot = st.ot_pool.tile([128, 512], bf16, name="ot")
        with nc.allow_low_precision(reason="bf16 attention output"):
            rls, rlbs = [], []
            for po in (oa, ob):
                rl = st.rl.tile([1, 512], f32, name="rl")
                nc.vector.reciprocal(rl[:], po[64:65, :])
                rls.append(rl)
            for rl in rls:
                rlb = st.rlb.tile([64, 512], f32, name="rlb")
                nc.gpsimd.partition_broadcast(rlb[:], rl[:])
                rlbs.append(rlb)
            for hl, po in ((0, oa), (1, ob)):
                nc.vector.tensor_mul(ot[64 * hl:64 * hl + 64, :],
                                     po[0:64, :], rlbs[hl][:])
        ot_tiles[(hp, j)] = ot


def make_inputs(x, W_attn, b_attn, W_proj, b_proj):
    """Host-side sharding: per-core input dicts."""
    import ml_dtypes
    x = np.asarray(x, np.float32)
    W_attn = np.asarray(W_attn, np.float32)
    b_attn = np.asarray(b_attn, np.float32)
    W_proj = np.asarray(W_proj, np.float32)
    b_proj = np.asarray(b_proj, np.float32)

    in_maps = []
    for core in range(N_CORES):
        b, g = divmod(core, 2)
        cols = np.concatenate([
            np.arange(CPC * g, CPC * g + CPC),
            C + np.arange(CPC * g, CPC * g + CPC),
            2 * C + np.arange(CPC * g, CPC * g + CPC)])
        wqkv = np.ascontiguousarray(W_attn[:, cols])
        bqk = b_attn[cols[:2 * CPC]].copy()           # q,k bias [1024]
        bqk[:CPC] *= SCALE                            # fold q-scale into bias
        bqkv = np.ascontiguousarray(bqk.reshape(8, 128).T)
        wp_f = W_proj[CPC * g:CPC * g + CPC, :]
        wp = np.ascontiguousarray(wp_f.astype(ml_dtypes.bfloat16))
        # fold the attention V-bias through the projection:
        # y += b_attn_v[g] @ W_proj[rows_g]  (+ b_proj once, on group 0)
        bv = b_attn[2 * C + CPC * g:2 * C + CPC * g + CPC]
        bpv = bv @ wp_f + (b_proj if g == 0 else 0.0)
        bpv = np.ascontiguousarray(bpv.astype(np.float32).reshape(8, 128).T)
        in_maps.append({
            "xb": np.ascontiguousarray(x[b].T),
            "wqkv": wqkv,
            "bqkv": bqkv,
            "wp": wp,
            "bp": bpv,
        })
    return in_maps


def unshard(results):
    """Combine per-core yT partials into [B, T, C] output."""
    out = np.empty((B, T, C), np.float32)
    for b in range(B):
        yt = results[2 * b]["yT"] + results[2 * b + 1]["yT"]
        out[b] = yt.T
    return out


_nc_cache = {}


def kernel(x, W_attn, b_attn, W_proj, b_proj):
    from concourse.bass_utils import run_bass_kernel_spmd
    if "nc" not in _nc_cache:
        _nc_cache["nc"] = build_nc(repeat=1)
    nc = _nc_cache["nc"]
    in_maps = make_inputs(x, W_attn, b_attn, W_proj, b_proj)
    res = run_bass_kernel_spmd(nc, in_maps, core_ids=list(range(N_CORES)),
                               trace=False)
    return unshard(res.results)
